# revision 19
# baseline (speedup 1.0000x reference)
"""TRN2 Bass kernel for nn_LocalAggregation (gnn_message_passing).

Reference computation (per batch b, point n, neighbor k):
    pn = p[idx[n,k]]; dp = pn - p[n]                        # [3]
    arg[a,t] = 50*dp[a] / 500^(t/32)      (a<3, t<32)       # 96 args
    pe = [sin(arg) interleaved cos(arg)] per reference channel order
    agg = (x[:, idx[n,k]] + 1) * pe                          # [192]
    h = [dp; agg];  y = (W h) * inv + add;  out = max_k relu(y)

Mapping onto 8 NeuronCores: core c -> batch b=c//2, point half h=c%2 (2048 pts).

Device pipeline per core (all matmuls bf16 with exactly-representable
selector weights; precision via bf16 hi/lo splits):
  - host builds a gather table TH [4096, 256] bf16 per batch:
      elems 0..95   = (x+1)_hi for "sin block" channels (a*64+t)
      elems 128..223= (x+1)_hi for "cos block" channels (a*64+32+t)
      elems 224..232= p split into 3 bf16 components (hi, mid, lo)
  - dma_gather(transpose=True) -> slab [128, 2, FG]: channel-major tiles
  - mm_pn: ones-selector lhsT over the 9 p-component partitions -> psD [99, F]
      (d replicated: partition i<96 -> d[a(i)], 96..98 -> d[a])
  - mm_pc: -ones-selector over broadcast pc components (pT 3-way split) -> accum
  - ACT: pe_sin = Sin(psD * s), pe_cos = Sin(psD * s + pi/2)  (scale AP = s)
  - DVE: agg = slab * pe (bf16, 2x mode); ACT: dp cast from psD
  - 4 bf16 matmuls (contraction 96 / 99, M-halves 128+64) -> psY [128, 1024]
  - DVE reduce max over k (32) -> [128, 2, 16]; ACT relu+bias -> out slab
"""

import os
import sys
import threading

import numpy as np

sys.path.insert(0, "/opt/trn_rl_repo")

import ml_dtypes

B, N, K, C = 4, 4096, 32, 192
FD = C // 6
EPS = 1e-5
NCORES = 8
NP = N // 2          # points per core
F = 512              # columns per sub-tile (16 points)
FG = 4096            # columns per gather slab (128 points)
NSUB = FG // F       # 8 sub-tiles per slab
NSLAB = NP * K // FG  # 16 slabs per core
PTS_SLAB = FG // K   # 128 points per slab
PTS_SUB = F // K     # 16 points per sub-tile

bf16 = ml_dtypes.bfloat16

_a96 = np.arange(96) // 32
_t96 = np.arange(96) % 32
C_SIN = _a96 * 64 + _t96          # orig x-channel for sin slot j
C_COS = _a96 * 64 + 32 + _t96     # orig x-channel for cos slot j
A99 = np.concatenate([_a96, np.arange(3)])  # axis index per psD partition

_dim_mat = np.power(np.float64(500.0), np.arange(FD, dtype=np.float64) / FD)
S96 = (50.0 / _dim_mat).astype(np.float32)[_t96]  # scale per arg slot
# turns-per-unit-d: q = (s/2pi)*d + 100; sin(arg) = sin(2pi*(q - round(q)))
SP96 = (S96.astype(np.float64) / (2 * np.pi)).astype(np.float32)
SP_COMPS = None  # filled lazily in build_weights via _split3
MAGIC = float(1.5 * 2.0**23)  # fp32 round-to-nearest via (q+M)-M


def _split3(x):
    """fp32 -> three bf16 components summing to ~fp32 precision."""
    h = x.astype(bf16)
    r = x - h.astype(np.float32)
    m = r.astype(bf16)
    l = (r - m.astype(np.float32)).astype(bf16)
    return h, m, l


def build_tables(p_b, x_b):
    """p_b [N,3] f32, x_b [C,N] f32 -> TH [N,256] bf16."""
    xp = x_b + np.float32(1.0)
    xp_hi = xp.astype(bf16)
    p3 = _split3(p_b)
    TH = np.zeros((N, 256), bf16)
    TH[:, 0:96] = xp_hi[C_SIN, :].T
    TH[:, 128:224] = xp_hi[C_COS, :].T
    comps9 = np.concatenate(p3, axis=1)  # [N, 9] (p_hi, p_mid, p_lo) x 3 axes
    for u in range(3):
        TH[:, 96 + 9 * u : 105 + 9 * u] = comps9
    return TH


def build_weights(W, gamma, beta, rmean, rvar):
    inv = (gamma / np.sqrt(rvar + EPS)).astype(np.float32)
    Wp = (W * inv[:, None]).astype(np.float32)   # [192, 195]
    add = (beta - rmean * inv).astype(np.float32)
    ly0 = Wp[:, 3 + C_SIN].T.astype(bf16)        # [96, 192]
    ly1 = np.zeros((99, 192), np.float32)
    # cos block negated: device computes -cos via sin(2pi*(|w| - 1/4))
    ly1[0:96] = -Wp[:, 3 + C_COS].T
    ly1[96:99] = Wp[:, 0:3].T
    ly1 = ly1.astype(bf16)
    # selector lhsTs.
    # w_pn is consumed as rhs slab[64:123] (rows 0..31 = x junk, rows 32..58 =
    # three copies of the 9 p components, copy u scaled by s'_u = split_u(s/2pi)).
    # w_pc mirrors it over the PT broadcast rows plus a +100 const row.
    sp = [c.astype(np.float32) for c in _split3(SP96)]
    w_pn = np.zeros((59, 99), np.float32)
    w_pc = np.zeros((28, 99), np.float32)
    for u in range(3):
        for va in range(9):
            a = va % 3
            sel = (A99[0:96] == a).astype(np.float32)
            w_pn[32 + 9 * u + va, 0:96] = sp[u] * sel
            w_pc[9 * u + va, 0:96] = -sp[u] * sel
    # dp rows (96..98): plain pn - pc from the u=0 copy, all three v comps
    for va in range(9):
        a = va % 3
        w_pn[32 + va, 96 + a] += 1.0
        w_pc[va, 96 + a] += -1.0
    w_pc[27, 0:96] = 100.0  # q shift (exact in bf16)
    svec = np.zeros((99, 1), np.float32)  # kept for interface compat (unused)
    badd = np.zeros((128, 2), np.float32)
    badd[:, 0] = add[0:128]
    badd[0:64, 1] = add[128:192]
    return dict(
        ly0=ly0,
        ly1=ly1,
        w_pn=w_pn.astype(bf16),
        w_pc=w_pc.astype(bf16),
        svec=svec,
        badd=badd,
    )


def wrap_idx(idx_core):
    """idx slice [NP, K] int -> [NSLAB, 128, FG//16] int16 wrapped for dma_gather."""
    flat = idx_core.reshape(-1).astype(np.int16)
    out = np.zeros((NSLAB, 128, FG // 16), np.int16)
    for g in range(NSLAB):
        sl = flat[g * FG : (g + 1) * FG].reshape(FG // 16, 16).T  # [16, FG//16]
        out[g] = np.tile(sl, (8, 1))
    return out


def _build_program():
    import concourse.bacc as bacc
    import concourse.bass as bass
    import concourse.mybir as mybir
    import concourse.tile as tile

    f32 = mybir.dt.float32
    bf = mybir.dt.bfloat16
    i16 = mybir.dt.int16
    AF = mybir.ActivationFunctionType

    nslab_run = int(os.environ.get("K_NSLAB", NSLAB))

    nc = bacc.Bacc("TRN2", target_bir_lowering=False, debug=False)
    TH = nc.dram_tensor("TH", [N, 256], bf, kind="ExternalInput")
    IDXW = nc.dram_tensor("IDXW", [NSLAB, 128, FG // 16], i16, kind="ExternalInput")
    CB = nc.dram_tensor("CB", [128, NP + 99 + 99 + 192 + 192], bf, kind="ExternalInput")
    CF = nc.dram_tensor("CF", [128, 3], f32, kind="ExternalInput")
    OUT = nc.dram_tensor("OUT", [192, NP], f32, kind="ExternalOutput")

    with tile.TileContext(nc) as tc:
        with (
            tc.tile_pool(name="const", bufs=1) as cp,
            tc.tile_pool(name="slab", bufs=2) as sp,
            tc.tile_pool(name="work", bufs=3) as wp,
            tc.tile_pool(name="outp", bufs=2) as op,
            tc.tile_pool(name="psd", bufs=2, space="PSUM") as ppd,
            tc.tile_pool(name="psy", bufs=2, space="PSUM") as ppy,
        ):
            cb = cp.tile([128, NP + 99 + 99 + 192 + 192], bf)
            nc.sync.dma_start(out=cb[:], in_=CB[:])
            o0, o1, o2, o3, o4 = NP, NP + 99, NP + 198, NP + 390, NP + 582
            pt = cb[0:28, 0:NP]
            w_pn = cb[64:123, o0:o1]
            w_pc = cb[0:28, o1:o2]
            ly0 = cb[0:96, o2:o3]
            ly1 = cb[0:99, o3:o4]
            cf = cp.tile([128, 3], f32)
            nc.sync.dma_start(out=cf[:], in_=CF[:])
            svec = cf[0:96, 0:1]
            badd = cf[:, 1:3]
            neghp = cp.tile([96, 1], f32)
            nc.gpsimd.memset(neghp[:], float(-np.pi / 2))

            for g in range(nslab_run):
                idxt = sp.tile([128, FG // 16], i16, tag="idx")
                nc.sync.dma_start(out=idxt[:], in_=IDXW[g])
                gch = int(os.environ.get("K_GCH", 512))
                ng = FG // gch
                slab = sp.tile([128, ng, 2, gch], bf, tag="slab")
                for j in range(ng):
                    nc.gpsimd.dma_gather(
                        slab[:, j, :, :],
                        TH[:],
                        idxt[:, j * (gch // 16) : (j + 1) * (gch // 16)],
                        gch,
                        gch,
                        256,
                        transpose=True,
                    )
                outs = op.tile([128, 2, PTS_SLAB], f32, tag="outs")
                for s in range(NSUB):
                    jj = (s * F) // gch
                    off = (s * F) % gch
                    cols = slice(off, off + F)
                    pt0 = g * PTS_SLAB + s * PTS_SUB
                    # d (replicated to 99 partitions) = pn - pc, fp32-exact
                    psd = ppd.tile([99, F], f32, tag="psd")
                    nc.tensor.matmul(
                        psd[:],
                        lhsT=w_pn,
                        rhs=slab[64:123, jj, 0, cols],
                        start=True,
                        stop=False,
                    )
                    pc_rhs = (
                        pt[:, pt0 : pt0 + PTS_SUB]
                        .rearrange("p (n o) -> p n o", o=1)
                        .to_broadcast([28, PTS_SUB, K])
                    )
                    nc.tensor.matmul(
                        psd[:], lhsT=w_pc, rhs=pc_rhs, start=False, stop=True
                    )
                    # psd rows 0..95 hold q = arg/(2pi) + 100.
                    # r = round(q) via (q + M) - M; w = q - r in [-0.5, 0.5]
                    rr = wp.tile([96, F], f32, tag="rr")
                    nc.vector.tensor_scalar(
                        rr[:],
                        psd[0:96, :],
                        MAGIC,
                        -MAGIC,
                        op0=mybir.AluOpType.add,
                        op1=mybir.AluOpType.add,
                    )
                    ww = wp.tile([96, F], f32, tag="ww")
                    nc.vector.tensor_tensor(
                        out=ww[:], in0=psd[0:96, :], in1=rr[:],
                        op=mybir.AluOpType.subtract,
                    )
                    # wc = |2pi*w| (ACT Abs); sin(wc - pi/2) = -cos(arg)
                    wc = wp.tile([96, F], f32, tag="wc")
                    nc.scalar.activation(wc[:], ww[:], AF.Abs, scale=float(2 * np.pi))
                    # pe0 = sin(2pi*w) = sin(arg); pe1 = -cos(arg) (ly1 negated)
                    pe = wp.tile([96, 2, F], bf, tag="pe")
                    nc.scalar.activation(
                        pe[:, 0, :], ww[:], AF.Sin, scale=float(2 * np.pi)
                    )
                    nc.scalar.activation(pe[:, 1, :], wc[:], AF.Sin, bias=neghp[:])
                    # agg = slab_x * pe ; dp cast into agg[96:99, 1, :]
                    agg = wp.tile([99, 2, F], bf, tag="agg")
                    nc.vector.tensor_tensor(
                        out=agg[0:96, :, :],
                        in0=slab[0:96, jj, :, cols],
                        in1=pe[:],
                        op=mybir.AluOpType.mult,
                    )
                    nc.scalar.copy(agg[96:99, 1, :], psd[96:99, :])
                    # y matmuls: psY [128, 1024] = two 512-col M-half blocks
                    psy = ppy.tile([128, 1024], f32, tag="psy")
                    nc.tensor.matmul(
                        psy[:, 0:512],
                        lhsT=ly0[:, 0:128],
                        rhs=agg[0:96, 0, :],
                        start=True,
                        stop=False,
                    )
                    nc.tensor.matmul(
                        psy[:, 0:512],
                        lhsT=ly1[:, 0:128],
                        rhs=agg[:, 1, :],
                        start=False,
                        stop=True,
                    )
                    nc.tensor.matmul(
                        psy[0:64, 512:1024],
                        lhsT=ly0[:, 128:192],
                        rhs=agg[0:96, 0, :],
                        start=True,
                        stop=False,
                    )
                    nc.tensor.matmul(
                        psy[0:64, 512:1024],
                        lhsT=ly1[:, 128:192],
                        rhs=agg[:, 1, :],
                        start=False,
                        stop=True,
                    )
                    # reduce max over k
                    red = wp.tile([128, 2, PTS_SUB], f32, tag="red")
                    nc.vector.tensor_reduce(
                        red[:, 0, :],
                        psy[:, 0:512].rearrange("p (n k) -> p n k", k=K),
                        axis=mybir.AxisListType.X,
                        op=mybir.AluOpType.max,
                    )
                    nc.vector.tensor_reduce(
                        red[0:64, 1, :],
                        psy[0:64, 512:1024].rearrange("p (n k) -> p n k", k=K),
                        axis=mybir.AxisListType.X,
                        op=mybir.AluOpType.max,
                    )
                    # relu + bias -> out slab
                    oc = slice(s * PTS_SUB, (s + 1) * PTS_SUB)
                    nc.scalar.activation(
                        outs[:, 0, oc], red[:, 0, :], AF.Relu, bias=badd[:, 0:1]
                    )
                    nc.scalar.activation(
                        outs[0:64, 1, oc], red[0:64, 1, :], AF.Relu, bias=badd[0:64, 1:2]
                    )
                nc.sync.dma_start(
                    out=OUT[0:128, g * PTS_SLAB : (g + 1) * PTS_SLAB],
                    in_=outs[:, 0, :],
                )
                nc.sync.dma_start(
                    out=OUT[128:192, g * PTS_SLAB : (g + 1) * PTS_SLAB],
                    in_=outs[0:64, 1, :],
                )
    nc.finalize()
    return nc


_PROGRAM = None
_PROGRAM_LOCK = threading.Lock()


def _get_program():
    global _PROGRAM
    with _PROGRAM_LOCK:
        if _PROGRAM is None:
            _PROGRAM = _build_program()
    return _PROGRAM


def make_in_maps(p, x, idx, W, gamma, beta, rmean, rvar):
    p = np.asarray(p, np.float32)
    x = np.asarray(x, np.float32)
    idx = np.asarray(idx)
    wd = build_weights(
        np.asarray(W, np.float32),
        np.asarray(gamma, np.float32),
        np.asarray(beta, np.float32),
        np.asarray(rmean, np.float32),
        np.asarray(rvar, np.float32),
    )
    o0, o1, o2, o3, o4 = NP, NP + 99, NP + 198, NP + 390, NP + 582
    CF = np.zeros((128, 3), np.float32)
    CF[:, 1:3] = wd["badd"]
    cb_base = np.zeros((128, o4), bf16)
    cb_base[64:123, o0:o1] = wd["w_pn"].astype(bf16)
    cb_base[0:28, o1:o2] = wd["w_pc"].astype(bf16)
    cb_base[0:96, o2:o3] = wd["ly0"]
    cb_base[0:99, o3:o4] = wd["ly1"]
    in_maps = []
    for b in range(B):
        TH = build_tables(p[b], x[b])
        for h in range(2):
            n0 = h * NP
            pT = p[b, n0 : n0 + NP].T  # [3, NP]
            comps = np.concatenate(_split3(pT), axis=0)  # [9, NP]
            PT = np.concatenate(
                [comps, comps, comps, np.ones((1, NP), bf16)], axis=0
            )  # [28, NP] bf16
            CB = cb_base.copy()
            CB[0:28, 0:NP] = PT
            in_maps.append(
                dict(
                    TH=TH,
                    IDXW=wrap_idx(idx[b, n0 : n0 + NP]),
                    CB=CB,
                    CF=CF,
                )
            )
    return in_maps


def kernel(p, x, idx, W, gamma, beta, rmean, rvar):
    from concourse.bass_utils import run_bass_kernel_spmd

    nc = _get_program()
    in_maps = make_in_maps(p, x, idx, W, gamma, beta, rmean, rvar)
    res = run_bass_kernel_spmd(nc, in_maps, list(range(NCORES)))
    out = np.zeros((B, C, N), np.float32)
    for c in range(NCORES):
        b, h = c // 2, c % 2
        out[b, :, h * NP : (h + 1) * NP] = res.results[c]["OUT"]
    return out


if __name__ == "__main__":
    # quick CoreSim check on a single core with small fake data
    pass


# revision 20
# speedup vs baseline: 1.0312x; 1.0312x over previous
"""TRN2 Bass kernel for nn_LocalAggregation (gnn_message_passing).

Reference computation (per batch b, point n, neighbor k):
    pn = p[idx[n,k]]; dp = pn - p[n]                        # [3]
    arg[a,t] = 50*dp[a] / 500^(t/32)      (a<3, t<32)       # 96 args
    pe = [sin(arg) interleaved cos(arg)] per reference channel order
    agg = (x[:, idx[n,k]] + 1) * pe                          # [192]
    h = [dp; agg];  y = (W h) * inv + add;  out = max_k relu(y)

Mapping onto 8 NeuronCores: core c -> batch b=c//2, point half h=c%2 (2048 pts).

Device pipeline per core (all matmuls bf16 with exactly-representable
selector weights; precision via bf16 hi/lo splits):
  - host builds a gather table TH [4096, 256] bf16 per batch:
      elems 0..95   = (x+1)_hi for "sin block" channels (a*64+t)
      elems 128..223= (x+1)_hi for "cos block" channels (a*64+32+t)
      elems 224..232= p split into 3 bf16 components (hi, mid, lo)
  - dma_gather(transpose=True) -> slab [128, 2, FG]: channel-major tiles
  - mm_pn: ones-selector lhsT over the 9 p-component partitions -> psD [99, F]
      (d replicated: partition i<96 -> d[a(i)], 96..98 -> d[a])
  - mm_pc: -ones-selector over broadcast pc components (pT 3-way split) -> accum
  - ACT: pe_sin = Sin(psD * s), pe_cos = Sin(psD * s + pi/2)  (scale AP = s)
  - DVE: agg = slab * pe (bf16, 2x mode); ACT: dp cast from psD
  - 4 bf16 matmuls (contraction 96 / 99, M-halves 128+64) -> psY [128, 1024]
  - DVE reduce max over k (32) -> [128, 2, 16]; ACT relu+bias -> out slab
"""

import os
import sys
import threading

import numpy as np

sys.path.insert(0, "/opt/trn_rl_repo")

import ml_dtypes

B, N, K, C = 4, 4096, 32, 192
FD = C // 6
EPS = 1e-5
NCORES = 8
NP = N // 2          # points per core
F = 512              # columns per sub-tile (16 points)
FG = 4096            # columns per gather slab (128 points)
NSUB = FG // F       # 8 sub-tiles per slab
NSLAB = NP * K // FG  # 16 slabs per core
PTS_SLAB = FG // K   # 128 points per slab
PTS_SUB = F // K     # 16 points per sub-tile

bf16 = ml_dtypes.bfloat16

_a96 = np.arange(96) // 32
_t96 = np.arange(96) % 32
C_SIN = _a96 * 64 + _t96          # orig x-channel for sin slot j
C_COS = _a96 * 64 + 32 + _t96     # orig x-channel for cos slot j
A99 = np.concatenate([_a96, np.arange(3)])  # axis index per psD partition

_dim_mat = np.power(np.float64(500.0), np.arange(FD, dtype=np.float64) / FD)
S96 = (50.0 / _dim_mat).astype(np.float32)[_t96]  # scale per arg slot
# turns-per-unit-d: q = (s/2pi)*d + 100; sin(arg) = sin(2pi*(q - round(q)))
SP96 = (S96.astype(np.float64) / (2 * np.pi)).astype(np.float32)
SP_COMPS = None  # filled lazily in build_weights via _split3
MAGIC = float(1.5 * 2.0**23)  # fp32 round-to-nearest via (q+M)-M


def _split3(x):
    """fp32 -> three bf16 components summing to ~fp32 precision."""
    h = x.astype(bf16)
    r = x - h.astype(np.float32)
    m = r.astype(bf16)
    l = (r - m.astype(np.float32)).astype(bf16)
    return h, m, l


def build_tables(p_b, x_b):
    """p_b [N,3] f32, x_b [C,N] f32 -> TH [N,256] bf16."""
    xp = x_b + np.float32(1.0)
    xp_hi = xp.astype(bf16)
    p3 = _split3(p_b)
    TH = np.zeros((N, 256), bf16)
    TH[:, 0:96] = xp_hi[C_SIN, :].T
    TH[:, 128:224] = xp_hi[C_COS, :].T
    comps9 = np.concatenate(p3, axis=1)  # [N, 9] (p_hi, p_mid, p_lo) x 3 axes
    for u in range(3):
        TH[:, 96 + 9 * u : 105 + 9 * u] = comps9
    return TH


def build_weights(W, gamma, beta, rmean, rvar):
    inv = (gamma / np.sqrt(rvar + EPS)).astype(np.float32)
    Wp = (W * inv[:, None]).astype(np.float32)   # [192, 195]
    add = (beta - rmean * inv).astype(np.float32)
    ly0 = Wp[:, 3 + C_SIN].T.astype(bf16)        # [96, 192]
    ly1 = np.zeros((99, 192), np.float32)
    # cos block negated: device computes -cos via sin(2pi*(|w| - 1/4))
    ly1[0:96] = -Wp[:, 3 + C_COS].T
    ly1[96:99] = Wp[:, 0:3].T
    ly1 = ly1.astype(bf16)
    # selector lhsTs.
    # w_pn is consumed as rhs slab[64:123] (rows 0..31 = x junk, rows 32..58 =
    # three copies of the 9 p components, copy u scaled by s'_u = split_u(s/2pi)).
    # w_pc mirrors it over the PT broadcast rows plus a +100 const row.
    sp = [c.astype(np.float32) for c in _split3(SP96)]
    w_pn = np.zeros((59, 99), np.float32)
    w_pc = np.zeros((28, 99), np.float32)
    for u in range(3):
        for va in range(9):
            a = va % 3
            sel = (A99[0:96] == a).astype(np.float32)
            w_pn[32 + 9 * u + va, 0:96] = sp[u] * sel
            w_pc[9 * u + va, 0:96] = -sp[u] * sel
    # dp rows (96..98): plain pn - pc from the u=0 copy, all three v comps
    for va in range(9):
        a = va % 3
        w_pn[32 + va, 96 + a] += 1.0
        w_pc[va, 96 + a] += -1.0
    w_pc[27, 0:96] = 100.0  # q shift (exact in bf16)
    svec = np.zeros((99, 1), np.float32)  # kept for interface compat (unused)
    badd = np.zeros((128, 2), np.float32)
    badd[:, 0] = add[0:128]
    badd[0:64, 1] = add[128:192]
    return dict(
        ly0=ly0,
        ly1=ly1,
        w_pn=w_pn.astype(bf16),
        w_pc=w_pc.astype(bf16),
        svec=svec,
        badd=badd,
    )


def wrap_idx(idx_core):
    """idx slice [NP, K] int -> [NSLAB, 128, FG//16] int16 wrapped for dma_gather."""
    flat = idx_core.reshape(-1).astype(np.int16)
    out = np.zeros((NSLAB, 128, FG // 16), np.int16)
    for g in range(NSLAB):
        sl = flat[g * FG : (g + 1) * FG].reshape(FG // 16, 16).T  # [16, FG//16]
        out[g] = np.tile(sl, (8, 1))
    return out


def _build_program():
    import concourse.bacc as bacc
    import concourse.bass as bass
    import concourse.mybir as mybir
    import concourse.tile as tile

    f32 = mybir.dt.float32
    bf = mybir.dt.bfloat16
    i16 = mybir.dt.int16
    AF = mybir.ActivationFunctionType

    nslab_run = int(os.environ.get("K_NSLAB", NSLAB))

    nc = bacc.Bacc("TRN2", target_bir_lowering=False, debug=False)
    TH = nc.dram_tensor("TH", [N, 256], bf, kind="ExternalInput")
    IDXW = nc.dram_tensor("IDXW", [NSLAB, 128, FG // 16], i16, kind="ExternalInput")
    CB = nc.dram_tensor("CB", [128, NP + 99 + 99 + 192 + 192], bf, kind="ExternalInput")
    CF = nc.dram_tensor("CF", [128, 3], f32, kind="ExternalInput")
    OUT = nc.dram_tensor("OUT", [192, NP], f32, kind="ExternalOutput")

    with tile.TileContext(nc) as tc:
        with (
            tc.tile_pool(name="const", bufs=1) as cp,
            tc.tile_pool(name="slab", bufs=2) as sp,
            tc.tile_pool(name="work", bufs=3) as wp,
            tc.tile_pool(name="outp", bufs=2) as op,
            tc.tile_pool(name="psd", bufs=2, space="PSUM") as ppd,
            tc.tile_pool(name="psy", bufs=2, space="PSUM") as ppy,
        ):
            cb = cp.tile([128, NP + 99 + 99 + 192 + 192], bf)
            nc.sync.dma_start(out=cb[:], in_=CB[:])
            o0, o1, o2, o3, o4 = NP, NP + 99, NP + 198, NP + 390, NP + 582
            pt = cb[0:28, 0:NP]
            w_pn = cb[64:123, o0:o1]
            w_pc = cb[0:28, o1:o2]
            ly0 = cb[0:96, o2:o3]
            ly1 = cb[0:99, o3:o4]
            cf = cp.tile([128, 3], f32)
            nc.sync.dma_start(out=cf[:], in_=CF[:])
            svec = cf[0:96, 0:1]
            badd = cf[:, 1:3]
            neghp = cp.tile([96, 1], f32)
            nc.gpsimd.memset(neghp[:], float(-np.pi / 2))
            mgc = cp.tile([96, 1], f32)
            nc.gpsimd.memset(mgc[:], MAGIC)

            for g in range(nslab_run):
                idxt = sp.tile([128, FG // 16], i16, tag="idx")
                nc.sync.dma_start(out=idxt[:], in_=IDXW[g])
                gch = int(os.environ.get("K_GCH", 512))
                ng = FG // gch
                slab = sp.tile([128, ng, 2, gch], bf, tag="slab")
                for j in range(ng):
                    nc.gpsimd.dma_gather(
                        slab[:, j, :, :],
                        TH[:],
                        idxt[:, j * (gch // 16) : (j + 1) * (gch // 16)],
                        gch,
                        gch,
                        256,
                        transpose=True,
                    )
                outs = op.tile([128, 2, PTS_SLAB], f32, tag="outs")
                for s in range(NSUB):
                    jj = (s * F) // gch
                    off = (s * F) % gch
                    cols = slice(off, off + F)
                    pt0 = g * PTS_SLAB + s * PTS_SUB
                    # d (replicated to 99 partitions) = pn - pc, fp32-exact
                    psd = ppd.tile([99, F], f32, tag="psd")
                    nc.tensor.matmul(
                        psd[:],
                        lhsT=w_pn,
                        rhs=slab[64:123, jj, 0, cols],
                        start=True,
                        stop=False,
                    )
                    pc_rhs = (
                        pt[:, pt0 : pt0 + PTS_SUB]
                        .rearrange("p (n o) -> p n o", o=1)
                        .to_broadcast([28, PTS_SUB, K])
                    )
                    nc.tensor.matmul(
                        psd[:], lhsT=w_pc, rhs=pc_rhs, start=False, stop=True
                    )
                    # psd rows 0..95 hold q = arg/(2pi) + 100.
                    # ACT's fp32 add rounds: t = fl(q + M) = M + round(q);
                    # GPSIMD: rr = t - M = round(q); DVE: w = q - rr.
                    tq = wp.tile([96, F], f32, tag="tq")
                    nc.scalar.activation(
                        tq[:], psd[0:96, :], AF.Identity, bias=mgc[:]
                    )
                    rr = wp.tile([96, F], f32, tag="rr")
                    nc.gpsimd.tensor_scalar(
                        rr[:], tq[:], -MAGIC, None, op0=mybir.AluOpType.add
                    )
                    ww = wp.tile([96, F], f32, tag="ww")
                    nc.vector.tensor_tensor(
                        out=ww[:], in0=psd[0:96, :], in1=rr[:],
                        op=mybir.AluOpType.subtract,
                    )
                    # wc = |2pi*w| (ACT Abs); sin(wc - pi/2) = -cos(arg)
                    wc = wp.tile([96, F], f32, tag="wc")
                    nc.scalar.activation(wc[:], ww[:], AF.Abs, scale=float(2 * np.pi))
                    # pe0 = sin(2pi*w) = sin(arg); pe1 = -cos(arg) (ly1 negated)
                    pe = wp.tile([96, 2, F], bf, tag="pe")
                    nc.scalar.activation(
                        pe[:, 0, :], ww[:], AF.Sin, scale=float(2 * np.pi)
                    )
                    nc.scalar.activation(pe[:, 1, :], wc[:], AF.Sin, bias=neghp[:])
                    # agg = slab_x * pe ; dp cast into agg[96:99, 1, :]
                    agg = wp.tile([99, 2, F], bf, tag="agg")
                    nc.vector.tensor_tensor(
                        out=agg[0:96, :, :],
                        in0=slab[0:96, jj, :, cols],
                        in1=pe[:],
                        op=mybir.AluOpType.mult,
                    )
                    nc.scalar.copy(agg[96:99, 1, :], psd[96:99, :])
                    # y matmuls: psY [128, 1024] = two 512-col M-half blocks
                    psy = ppy.tile([128, 1024], f32, tag="psy")
                    nc.tensor.matmul(
                        psy[:, 0:512],
                        lhsT=ly0[:, 0:128],
                        rhs=agg[0:96, 0, :],
                        start=True,
                        stop=False,
                    )
                    nc.tensor.matmul(
                        psy[:, 0:512],
                        lhsT=ly1[:, 0:128],
                        rhs=agg[:, 1, :],
                        start=False,
                        stop=True,
                    )
                    nc.tensor.matmul(
                        psy[0:64, 512:1024],
                        lhsT=ly0[:, 128:192],
                        rhs=agg[0:96, 0, :],
                        start=True,
                        stop=False,
                    )
                    nc.tensor.matmul(
                        psy[0:64, 512:1024],
                        lhsT=ly1[:, 128:192],
                        rhs=agg[:, 1, :],
                        start=False,
                        stop=True,
                    )
                    # reduce max over k
                    red = wp.tile([128, 2, PTS_SUB], f32, tag="red")
                    nc.vector.tensor_reduce(
                        red[:, 0, :],
                        psy[:, 0:512].rearrange("p (n k) -> p n k", k=K),
                        axis=mybir.AxisListType.X,
                        op=mybir.AluOpType.max,
                    )
                    nc.vector.tensor_reduce(
                        red[0:64, 1, :],
                        psy[0:64, 512:1024].rearrange("p (n k) -> p n k", k=K),
                        axis=mybir.AxisListType.X,
                        op=mybir.AluOpType.max,
                    )
                    # relu + bias -> out slab
                    oc = slice(s * PTS_SUB, (s + 1) * PTS_SUB)
                    nc.scalar.activation(
                        outs[:, 0, oc], red[:, 0, :], AF.Relu, bias=badd[:, 0:1]
                    )
                    nc.scalar.activation(
                        outs[0:64, 1, oc], red[0:64, 1, :], AF.Relu, bias=badd[0:64, 1:2]
                    )
                nc.sync.dma_start(
                    out=OUT[0:128, g * PTS_SLAB : (g + 1) * PTS_SLAB],
                    in_=outs[:, 0, :],
                )
                nc.sync.dma_start(
                    out=OUT[128:192, g * PTS_SLAB : (g + 1) * PTS_SLAB],
                    in_=outs[0:64, 1, :],
                )
    nc.finalize()
    return nc


_PROGRAM = None
_PROGRAM_LOCK = threading.Lock()


def _get_program():
    global _PROGRAM
    with _PROGRAM_LOCK:
        if _PROGRAM is None:
            _PROGRAM = _build_program()
    return _PROGRAM


def make_in_maps(p, x, idx, W, gamma, beta, rmean, rvar):
    p = np.asarray(p, np.float32)
    x = np.asarray(x, np.float32)
    idx = np.asarray(idx)
    wd = build_weights(
        np.asarray(W, np.float32),
        np.asarray(gamma, np.float32),
        np.asarray(beta, np.float32),
        np.asarray(rmean, np.float32),
        np.asarray(rvar, np.float32),
    )
    o0, o1, o2, o3, o4 = NP, NP + 99, NP + 198, NP + 390, NP + 582
    CF = np.zeros((128, 3), np.float32)
    CF[:, 1:3] = wd["badd"]
    cb_base = np.zeros((128, o4), bf16)
    cb_base[64:123, o0:o1] = wd["w_pn"].astype(bf16)
    cb_base[0:28, o1:o2] = wd["w_pc"].astype(bf16)
    cb_base[0:96, o2:o3] = wd["ly0"]
    cb_base[0:99, o3:o4] = wd["ly1"]
    in_maps = []
    for b in range(B):
        TH = build_tables(p[b], x[b])
        for h in range(2):
            n0 = h * NP
            pT = p[b, n0 : n0 + NP].T  # [3, NP]
            comps = np.concatenate(_split3(pT), axis=0)  # [9, NP]
            PT = np.concatenate(
                [comps, comps, comps, np.ones((1, NP), bf16)], axis=0
            )  # [28, NP] bf16
            CB = cb_base.copy()
            CB[0:28, 0:NP] = PT
            in_maps.append(
                dict(
                    TH=TH,
                    IDXW=wrap_idx(idx[b, n0 : n0 + NP]),
                    CB=CB,
                    CF=CF,
                )
            )
    return in_maps


def kernel(p, x, idx, W, gamma, beta, rmean, rvar):
    from concourse.bass_utils import run_bass_kernel_spmd

    nc = _get_program()
    in_maps = make_in_maps(p, x, idx, W, gamma, beta, rmean, rvar)
    res = run_bass_kernel_spmd(nc, in_maps, list(range(NCORES)))
    out = np.zeros((B, C, N), np.float32)
    for c in range(NCORES):
        b, h = c // 2, c % 2
        out[b, :, h * NP : (h + 1) * NP] = res.results[c]["OUT"]
    return out


if __name__ == "__main__":
    # quick CoreSim check on a single core with small fake data
    pass
